# revision 16
# baseline (speedup 1.0000x reference)
"""Trainium2 Bass kernel for nn_ConvAttLIF (conv3x3 + temporal attention + LIF scan).

Sharding: data-parallel over batch B=16 across 8 NeuronCores (2 samples/core).

Conv: frames host-padded into a PW=33 flat layout (adjacent padded rows SHARE
one zero column: index(r,c) = 33r + c, so taps are o = 33dy+dx and row chunks
have only 1/33 junk columns). Main pass in f16 (1 cyc/col, half the DMA bytes
of f32): the 9 taps are covered by 5 matmuls using two shifted tile layouts:
  xm (lower=x, upper=x[+33])  -> K=128 pairs at windows a in {-33,-32,-1}
                                 covering taps {(-33,0),(-32,1),(-1,32)}
  xs (lower=x, upper=x[+1])   -> K=128 pair at window 33 covering (33,34)
  xm lower only               -> K=64 single for tap -34
The f16 cross terms xhi*wlo + xlo*whi are restored by an fp8 e4m3 DoubleRow
pass (0.5 cyc/col): xc planes (xhi8 | xhi8[+33]) and (xlo8*2^10 | ...) give 3
full pair-DRs + 3 half DRs, accumulated at scale 2^16 into psB and rescaled by
the ACT copy. Host flip-sim of this scheme: 3/41.9M flips (gate is 2e-2).

Epilogue y = (psA + bias) + yB runs on the otherwise-idle Pool (gpsimd)
engine with accum_out producing the avg-pool sums; the frame max stays on DVE
(2-operand tensor_scalar runs in the 2x DVE mode).

LIF scan (v-space, v = u/att): v = g*bc0 + y (STT), spike out = Sign(v - thr)
on ACT straight to fp8, g = v*[v<thr] via (v is_lt thr)*v (STT) -- the gate
reads v directly so ACT is off the serial chain. The trailing sample's scan
(nothing left to overlap) is column-split across DVE and Pool so both engines
advance the recurrence in parallel.

kernel(**inputs) takes the FULL unsharded inputs, returns the FULL output.
"""
import sys

sys.path.insert(0, "/opt/trn_rl_repo")

import numpy as np
import ml_dtypes
import concourse.bass as bass
import concourse.bacc as bacc
import concourse.tile as tile
import concourse.mybir as mybir
from concourse.bass_utils import run_bass_kernel_spmd

F32 = mybir.dt.float32
F16 = mybir.dt.float16
FP8 = mybir.dt.float8e4
AF = mybir.ActivationFunctionType
OP = mybir.AluOpType
DR = mybir.MatmulPerfMode.DoubleRow
E4 = ml_dtypes.float8_e4m3

B, T, CIN, H, W = 16, 20, 64, 32, 32
CH = 128
N_CORES = 8
BPC = B // N_CORES
ALPHA, VTH = 0.3, 0.6
HW = H * W                      # 1024
PW = H + 1                      # 33: shared zero column between padded rows
XOFF = PW + 1                   # buf[XOFF + 33r + c] = padded cell (r, c)
XW = 1126                       # sbuf row: covers flat idx -34 .. 1091
XLEN = XW + PW + 1              # host buffer, so +33-shifted reads stay in range
NY = 24                         # y-tile ring size

# output row chunks (psum bank holds 512 f32: rows*33 <= 512)
CHUNKS = [(0, 11), (11, 22), (22, 32)]
PAIR_A = [-33, -32, -1]         # xm windows: tap a (lower) + tap a+33 (upper)
XS_A = 33                       # xs window: tap 33 (lower) + tap 34 (upper)
SINGLE_A = -34                  # K=64 single tap
CORR_A = PAIR_A + [SINGLE_A, 33, 34]   # DR windows (last 3 lower-half only)
S_XLO, S_WLO, S_WHI = 10, 16, 6  # fp8 scales: xlo*2^10, wlo*2^16, whi*2^6
TAILD = 760                      # tail column split: DVE gets [0:760], Pool rest


def _build_program():
    nc = bacc.Bacc("TRN2", target_bir_lowering=False, debug=False,
                   num_devices=N_CORES)

    xm_d = nc.dram_tensor("xm", [BPC, T, 128, XW], F16,
                          kind="ExternalInput").ap()
    xs_d = nc.dram_tensor("xs", [BPC, T, 128, XW], F16,
                          kind="ExternalInput").ap()
    xc_d = nc.dram_tensor("xc", [BPC, T, 128, 2, XW], FP8,
                          kind="ExternalInput").ap()
    wmain_d = nc.dram_tensor("wmain", [128, 5 * 128], F16,
                             kind="ExternalInput").ap()
    wcorr_d = nc.dram_tensor("wcorr", [128, 6, 2, 128], FP8,
                             kind="ExternalInput").ap()
    bias_d = nc.dram_tensor("biasv", [128, 1], F32, kind="ExternalInput").ap()
    w1t_d = nc.dram_tensor("w1t", [T, 5], F32, kind="ExternalInput").ap()
    w2t_d = nc.dram_tensor("w2t", [5, T], F32, kind="ExternalInput").ap()
    ident_d = nc.dram_tensor("ident", [128, 128], F32, kind="ExternalInput").ap()
    spk = nc.dram_tensor("spk", [BPC, T, CH, HW], FP8,
                         kind="ExternalOutput").ap()

    with tile.TileContext(nc) as tc:
        with tc.tile_pool(name="sb", bufs=1) as P1, \
             tc.tile_pool(name="scr", bufs=2) as P2, \
             tc.tile_pool(name="so", bufs=3) as P3, \
             tc.tile_pool(name="ps", bufs=1, space="PSUM") as PP:

            wmain = P1.tile([128, 5 * 128], F16, tag="wmain", name="wmain")
            nc.sync.dma_start(wmain[:], wmain_d[:])
            wcorr = P1.tile([128, 6, 2, 128], FP8, tag="wcorr", name="wcorr")
            nc.sync.dma_start(wcorr[:], wcorr_d[:])
            biast = P1.tile([128, 1], F32, tag="biasv", name="biasv")
            nc.sync.dma_start(biast[:], bias_d[:])
            w1t_s = P1.tile([T, 5], F32, tag="w1t", name="w1t")
            nc.sync.dma_start(w1t_s[:], w1t_d[:])
            w2t_s = P1.tile([5, T], F32, tag="w2t", name="w2t")
            nc.sync.dma_start(w2t_s[:], w2t_d[:])
            ident = P1.tile([128, 128], F32, tag="ident", name="ident")
            nc.sync.dma_start(ident[:], ident_d[:])
            ones_t = P1.tile([1, 128], F32, tag="ones", name="ones")
            nc.vector.memset(ones_t[:], 1.0)

            ys = [P1.tile([128, HW], F32, tag=f"y{i}", name=f"y{i}")
                  for i in range(NY)]
            xms = [P1.tile([128, XW], F16, tag=f"xm{i}", name=f"xm{i}")
                   for i in range(3)]
            xss = [P1.tile([128, XW], F16, tag=f"xs{i}", name=f"xs{i}")
                   for i in range(3)]
            xcs = [P1.tile([128, 2, XW], FP8, tag=f"xc{i}", name=f"xc{i}")
                   for i in range(3)]
            g_t = P1.tile([128, HW], F32, tag="g", name="g")
            mscr = P1.tile([128, HW], F32, tag="mscr", name="mscr")
            # per-frame stats: 3 chunk yA sums + interior max (yB's ~2^-12
            # relative contribution to the attention avg is dropped)
            s_st = [P1.tile([128, 4 * T], F32, tag=f"S{s}", name=f"S{s}")
                    for s in range(BPC)]
            # per-step scalars: bc0 = a*att[t-1]/att[t], -thr, +thr
            bc = [P1.tile([128, 3 * T], F32, tag=f"bc{s}", name=f"bc{s}")
                  for s in range(BPC)]

            def conv_frame(s, t):
                f = s * T + t
                xm = xms[f % 3]
                nc.sync.dma_start(xm[:], xm_d[s, t])
                xs = xss[f % 3]
                nc.sync.dma_start(xs[:], xs_d[s, t])
                xc = xcs[f % 3]
                nc.scalar.dma_start(xc[:], xc_d[s, t])

                y = ys[f % NY]
                y2 = y.rearrange("p (r c) -> p r c", c=W)
                S = s_st[s]
                for c, (r0, r1) in enumerate(CHUNKS):
                    rows = r1 - r0
                    n = rows * PW
                    wbase = XOFF + r0 * PW
                    psA = PP.tile([128, 512], F32, tag=f"psA{c}",
                                  name=f"psA{c}")
                    psB = PP.tile([128, 512], F32, tag=f"psB{c}",
                                  name=f"psB{c}")
                    for j, a in enumerate(PAIR_A):
                        nc.tensor.matmul(
                            psA[:, 0:n], wmain[:, j * 128:(j + 1) * 128],
                            xm[:, wbase + a:wbase + a + n],
                            start=(j == 0), stop=False)
                    nc.tensor.matmul(
                        psA[:, 0:n], wmain[:, 3 * 128:4 * 128],
                        xs[:, wbase + XS_A:wbase + XS_A + n],
                        start=False, stop=False)
                    nc.tensor.matmul(
                        psA[:, 0:n], wmain[0:64, 4 * 128:5 * 128],
                        xm[0:64, wbase + SINGLE_A:wbase + SINGLE_A + n],
                        start=False, stop=True)
                    for j, a in enumerate(CORR_A):
                        nc.tensor.matmul(
                            psB[:, 0:n], wcorr[:, j, :, :],
                            xc[:, :, wbase + a:wbase + a + n],
                            perf_mode=DR, start=(j == 0), stop=(j == 5))
                    # interior views [p][row][col 0:32] (col 32 is shared pad)
                    pAv = psA[:, 0:n].rearrange(
                        "p (r w) -> p r w", w=PW)[:, :, 0:32]
                    pBv = psB[:, 0:n].rearrange(
                        "p (r w) -> p r w", w=PW)[:, :, 0:32]
                    # GPSIMD can't read PSUM and only runs TT/TS: ACT drains
                    # both psums to SBUF (folding bias / 2^-16 scale, accum
                    # sums for the attention avg), Pool adds them into y.
                    yA = P2.tile([128, 11 * W], F32, tag="yA", name="yA")
                    yA3 = yA[:, 0:rows * W].rearrange("p (r w) -> p r w", w=W)
                    nc.scalar.activation(yA3[:], pAv, AF.Identity,
                                         bias=biast[:, 0:1],
                                         accum_out=S[:, c * T + t:c * T + t + 1])
                    yB = P2.tile([128, 11 * W], F32, tag="yB", name="yB")
                    yB3 = yB[:, 0:rows * W].rearrange("p (r w) -> p r w", w=W)
                    nc.scalar.activation(yB3[:], pBv, AF.Copy,
                                         scale=2.0 ** -S_WLO)
                    nc.gpsimd.tensor_tensor(y2[:, r0:r1, :], yA3[:], yB3[:],
                                            op=OP.add)
                nc.vector.tensor_scalar(
                    mscr[:], y[:], -3.0e38, None, op0=OP.max, op1=OP.max,
                    accum_out=S[:, 3 * T + t:3 * T + t + 1])

            def attention(s):
                S = s_st[s]
                stot = P2.tile([128, T], F32, tag="stot", name="stot")
                nc.vector.tensor_tensor(stot[:], S[:, 0:T], S[:, T:2 * T],
                                        op=OP.add)
                nc.vector.tensor_tensor(stot[:], stot[:], S[:, 2 * T:3 * T],
                                        op=OP.add)
                psTs = PP.tile([T, 128], F32, tag="pT0", name="psTs")
                psTm = PP.tile([T, 128], F32, tag="pT1", name="psTm")
                nc.tensor.transpose(psTs[:], stot[:], ident[:])
                nc.tensor.transpose(psTm[:], S[:, 3 * T:4 * T], ident[:])
                att_in = P2.tile([T, 2], F32, tag="att_in", name="att_in")
                tmp = P2.tile([T, 1], F32, tag="att_tmp", name="att_tmp")
                nc.vector.reduce_sum(tmp[:], psTs[:], axis=mybir.AxisListType.X)
                nc.vector.tensor_scalar_mul(att_in[:, 0:1], tmp[:],
                                            1.0 / (CH * HW))
                nc.vector.reduce_max(att_in[:, 1:2], psTm[:],
                                     axis=mybir.AxisListType.X)
                ps5 = PP.tile([5, 2], F32, tag="pT0", name="ps5")
                nc.tensor.matmul(ps5[:], w1t_s[:], att_in[:], start=True,
                                 stop=True)
                h5 = P2.tile([5, 2], F32, tag="h5", name="h5")
                nc.scalar.activation(h5[:], ps5[:], AF.Relu)
                ps20 = PP.tile([T, 2], F32, tag="pT1", name="ps20")
                nc.tensor.matmul(ps20[:], w2t_s[:], h5[:], start=True, stop=True)
                a20 = P2.tile([T, 2], F32, tag="a20", name="a20")
                nc.scalar.activation(a20[:], ps20[:], AF.Copy)
                attp = P2.tile([T, 1], F32, tag="attp", name="attp")
                nc.vector.tensor_tensor(attp[:], a20[:, 0:1], a20[:, 1:2],
                                        op=OP.add)
                # sigmoid via exp + reciprocal; ab2 = [att | 1/att]
                expz = P2.tile([T, 1], F32, tag="expz", name="expz")
                nc.scalar.activation(expz[:], attp[:], AF.Exp, scale=-1.0)
                ab2 = P2.tile([T, 2], F32, tag="ab2", name="ab2")
                nc.vector.tensor_scalar_add(ab2[:, 1:2], expz[:], 1.0)
                nc.vector.reciprocal(ab2[:, 0:1], ab2[:, 1:2])
                # transpose att / 1/att to free-dim [1, T] vectors (PSUM
                # reads must start at partition 0, so two separate tiles)
                psTa = PP.tile([1, T], F32, tag="pT0", name="psTa")
                nc.tensor.transpose(psTa[:], ab2[:, 0:1], ident[0:T, 0:T])
                psTb = PP.tile([1, T], F32, tag="pT1", name="psTb")
                nc.tensor.transpose(psTb[:], ab2[:, 1:2], ident[0:T, 0:T])
                attf = P2.tile([1, T + 1], F32, tag="attf", name="attf")
                nc.vector.tensor_copy(attf[0:1, 1:T + 1], psTa[0:1, 0:T])
                nc.vector.tensor_copy(attf[0:1, 0:1], psTa[0:1, 0:1])
                rhs_bc = P2.tile([1, 3 * T], F32, tag="rhs_bc", name="rhs_bc")
                nc.vector.scalar_tensor_tensor(
                    rhs_bc[0:1, 0:T], attf[0:1, 0:T], ALPHA, psTb[0:1, 0:T],
                    op0=OP.mult, op1=OP.mult)
                nc.vector.tensor_scalar_mul(rhs_bc[0:1, T:2 * T],
                                            psTb[0:1, 0:T], -VTH)
                nc.vector.tensor_scalar_mul(rhs_bc[0:1, 2 * T:3 * T],
                                            psTb[0:1, 0:T], VTH)
                ps_bc = PP.tile([128, 3 * T], F32, tag="pT0", name="ps_bc")
                nc.tensor.matmul(ps_bc[:], ones_t[:], rhs_bc[:], start=True,
                                 stop=True)
                nc.scalar.activation(bc[s][:], ps_bc[:], AF.Copy)

            def scan_step(s, t, tail=False):
                f = s * T + t
                y = ys[f % NY]
                if t == 0:
                    vsrc = y
                else:
                    v = P2.tile([128, HW], F32, tag="v", name="v")
                    vsrc = v
                spm = P3.tile([128, HW], FP8, tag="spm", name="spm")
                if tail:
                    # column-split the serial chain: DVE runs [0:TAILD] with
                    # STT ops; Pool (no STT support) runs [TAILD:] with a
                    # TS + 2xTT chain that keeps g pre-scaled by the next
                    # step's bc0 so the v-update is a plain TT add. Spike out
                    # on ACT (idle in the tail), split to decouple the chains.
                    slD = slice(0, TAILD)
                    slP = slice(TAILD, HW)
                    if t != 0:
                        nc.vector.scalar_tensor_tensor(
                            vsrc[:, slD], g_t[:, slD], bc[s][:, t:t + 1],
                            y[:, slD], op0=OP.mult, op1=OP.add)
                        nc.gpsimd.tensor_tensor(vsrc[:, slP], g_t[:, slP],
                                                y[:, slP], op=OP.add)
                    nc.scalar.activation(
                        spm[:, slD], vsrc[:, slD], AF.Sign,
                        bias=bc[s][:, T + t:T + t + 1])
                    nc.scalar.activation(
                        spm[:, slP], vsrc[:, slP], AF.Sign,
                        bias=bc[s][:, T + t:T + t + 1])
                    if t != T - 1:
                        nc.vector.scalar_tensor_tensor(
                            g_t[:, slD], vsrc[:, slD],
                            bc[s][:, 2 * T + t:2 * T + t + 1],
                            vsrc[:, slD], op0=OP.is_lt, op1=OP.mult)
                        # gp = [v < thr] * bc0[t+1]; g = gp * v
                        gp = P2.tile([128, HW - TAILD], F32, tag="gp",
                                     name="gp")
                        nc.gpsimd.tensor_scalar(
                            gp[:], vsrc[:, slP],
                            bc[s][:, 2 * T + t:2 * T + t + 1],
                            bc[s][:, t + 1:t + 2], op0=OP.is_lt, op1=OP.mult)
                        nc.gpsimd.tensor_tensor(g_t[:, slP], gp[:],
                                                vsrc[:, slP], op=OP.mult)
                else:
                    # overlapped with conv: all on DVE (spike via 2-operand
                    # compare in the 2x DVE mode; ACT is busy draining psums)
                    if t != 0:
                        nc.vector.scalar_tensor_tensor(
                            vsrc[:], g_t[:], bc[s][:, t:t + 1], y[:],
                            op0=OP.mult, op1=OP.add)
                    nc.vector.tensor_single_scalar(
                        spm[:], vsrc[:], bc[s][:, 2 * T + t:2 * T + t + 1],
                        OP.is_ge)
                    if t != T - 1:
                        nc.vector.scalar_tensor_tensor(
                            g_t[:], vsrc[:],
                            bc[s][:, 2 * T + t:2 * T + t + 1],
                            vsrc[:], op0=OP.is_lt, op1=OP.mult)
                nc.scalar.dma_start(spk[s, t], spm[:])

            for t in range(T):
                conv_frame(0, t)
            attention(0)
            for t in range(T):
                scan_step(0, t)
                conv_frame(1, t)
            attention(1)
            for t in range(T):
                scan_step(1, t, tail=True)

    nc.compile()
    return nc


def _prep_host_inputs(conv_w, conv_b, mlp_w1, mlp_w2):
    whi = conv_w.astype(np.float16).astype(np.float32)   # [128,64,3,3]
    wlo = (conv_w - whi).astype(np.float32)

    def tapT(w, o):
        # o = dy*33 + dx with dy,dx in {-1,0,1}
        dy = int(np.round(o / PW))
        dx = o - PW * dy
        assert dy in (-1, 0, 1) and dx in (-1, 0, 1), o
        return np.ascontiguousarray(w[:, :, dy + 1, dx + 1].T)

    wmain = np.zeros((128, 5 * 128), np.float32)
    for j, a in enumerate(PAIR_A):
        wmain[0:64, j * 128:(j + 1) * 128] = tapT(whi, a)
        wmain[64:128, j * 128:(j + 1) * 128] = tapT(whi, a + PW)
    wmain[0:64, 3 * 128:4 * 128] = tapT(whi, 33)
    wmain[64:128, 3 * 128:4 * 128] = tapT(whi, 34)
    wmain[0:64, 4 * 128:5 * 128] = tapT(whi, SINGLE_A)

    wlo16 = wlo * np.float32(2.0 ** S_WLO)
    whi6 = whi * np.float32(2.0 ** S_WHI)
    wcorr = np.zeros((128, 6, 2, 128), np.float32)
    for j, a in enumerate(CORR_A):
        wcorr[0:64, j, 0, :] = tapT(wlo16, a)
        wcorr[0:64, j, 1, :] = tapT(whi6, a)
        if j < 3:
            wcorr[64:128, j, 0, :] = tapT(wlo16, a + PW)
            wcorr[64:128, j, 1, :] = tapT(whi6, a + PW)
    return {
        "wmain": wmain.astype(np.float16),
        "wcorr": wcorr.astype(E4),
        "biasv": np.ascontiguousarray(conv_b.reshape(128, 1), np.float32),
        "w1t": np.ascontiguousarray(mlp_w1.T).astype(np.float32),
        "w2t": np.ascontiguousarray(mlp_w2.T).astype(np.float32),
        "ident": np.eye(128, dtype=np.float32),
    }


def _shard_inputs(data):
    """data [BPC,T,64,32,32] -> xm/xs [BPC,T,128,XW] f16, xc [...,2,XW] e4m3.

    Flat PW=33 layout: buf[XOFF + 33r + c] = padded(r, c); interior r,c in
    [0,32); all pad cells (r or c = -1 or 32) collapse onto shared zeros.
    """
    lead = data.shape[:2]
    xp = np.zeros(lead + (CIN, XLEN), np.float32)
    idx = XOFF + PW * np.arange(H)[:, None] + np.arange(W)[None, :]
    xp[..., idx.reshape(-1)] = data.reshape(lead + (CIN, HW))
    xhi = xp.astype(np.float16)
    xlo10 = ((xp - xhi.astype(np.float32)) * np.float32(2.0 ** S_XLO))
    xm = np.empty(lead + (128, XW), np.float16)
    xm[..., 0:64, :] = xhi[..., 0:XW]
    xm[..., 64:128, :] = xhi[..., PW:PW + XW]
    xs = np.empty(lead + (128, XW), np.float16)
    xs[..., 0:64, :] = xhi[..., 0:XW]
    xs[..., 64:128, :] = xhi[..., 1:1 + XW]
    xhi8 = xhi.astype(E4)
    xlo8 = xlo10.astype(E4)
    xc = np.empty(lead + (128, 2, XW), E4)
    xc[..., 0:64, 0, :] = xhi8[..., 0:XW]
    xc[..., 0:64, 1, :] = xlo8[..., 0:XW]
    xc[..., 64:128, 0, :] = xhi8[..., PW:PW + XW]
    xc[..., 64:128, 1, :] = xlo8[..., PW:PW + XW]
    return xm, xs, xc


_CACHED = {}


def make_in_maps(data, conv_w, conv_b, mlp_w1, mlp_w2):
    data = np.ascontiguousarray(data, np.float32)
    common = _prep_host_inputs(np.asarray(conv_w, np.float32),
                               np.asarray(conv_b, np.float32),
                               np.asarray(mlp_w1, np.float32),
                               np.asarray(mlp_w2, np.float32))
    in_maps = []
    for c in range(N_CORES):
        m = dict(common)
        xm, xs, xc = _shard_inputs(data[c * BPC:(c + 1) * BPC])
        m["xm"] = xm
        m["xs"] = xs
        m["xc"] = xc
        in_maps.append(m)
    return in_maps


def kernel(data, conv_w, conv_b, mlp_w1, mlp_w2):
    if "prog" not in _CACHED:
        _CACHED["prog"] = _build_program()
    nc = _CACHED["prog"]
    in_maps = make_in_maps(data, conv_w, conv_b, mlp_w1, mlp_w2)
    res = run_bass_kernel_spmd(nc, in_maps, list(range(N_CORES)))
    out = np.concatenate(
        [np.asarray(res.results[c]["spk"]).astype(np.float32)
         for c in range(N_CORES)], axis=0)
    out = (out > 0).astype(np.float32)
    return out.reshape(B, T, CH, H, W)


# revision 24
# speedup vs baseline: 1.0110x; 1.0110x over previous
"""Trainium2 Bass kernel for nn_ConvAttLIF (conv3x3 + temporal attention + LIF scan).

Sharding: data-parallel over batch B=16 across 8 NeuronCores (2 samples/core).

Conv: frames host-padded into a PW=33 flat layout (adjacent padded rows SHARE
one zero column: index(r,c) = 33r + c, so taps are o = 33dy+dx and row chunks
have only 1/33 junk columns). Main pass in f16 (1 cyc/col, half the DMA bytes
of f32): the 9 taps are covered by 5 matmuls using two shifted tile layouts:
  xm (lower=x, upper=x[+33])  -> K=128 pairs at windows a in {-33,-32,-1}
                                 covering taps {(-33,0),(-32,1),(-1,32)}
  xs (lower=x, upper=x[+1])   -> K=128 pair at window 33 covering (33,34)
  xm lower only               -> K=64 single for tap -34
The f16 cross terms xhi*wlo + xlo*whi are restored by an fp8 e4m3 DoubleRow
pass (0.5 cyc/col): xc planes (xhi8 | xhi8[+33]) and (xlo8*2^10 | ...) give 3
full pair-DRs + 3 half DRs, accumulated at scale 2^16 into psB and rescaled by
the ACT copy. Host flip-sim of this scheme: 3/41.9M flips (gate is 2e-2).

Epilogue y = (psA + bias) + yB runs on the otherwise-idle Pool (gpsimd)
engine with accum_out producing the avg-pool sums; the frame max stays on DVE
(2-operand tensor_scalar runs in the 2x DVE mode).

LIF scan (v-space, v = u/att): v = g*bc0 + y (STT), spike out = Sign(v - thr)
on ACT straight to fp8, g = v*[v<thr] via (v is_lt thr)*v (STT) -- the gate
reads v directly so ACT is off the serial chain. The trailing sample's scan
(nothing left to overlap) is column-split across DVE and Pool so both engines
advance the recurrence in parallel.

kernel(**inputs) takes the FULL unsharded inputs, returns the FULL output.
"""
import sys

sys.path.insert(0, "/opt/trn_rl_repo")

import numpy as np
import ml_dtypes
import concourse.bass as bass
import concourse.bacc as bacc
import concourse.tile as tile
import concourse.mybir as mybir
from concourse.bass_utils import run_bass_kernel_spmd

F32 = mybir.dt.float32
F16 = mybir.dt.float16
FP8 = mybir.dt.float8e4
AF = mybir.ActivationFunctionType
OP = mybir.AluOpType
DR = mybir.MatmulPerfMode.DoubleRow
E4 = ml_dtypes.float8_e4m3

B, T, CIN, H, W = 16, 20, 64, 32, 32
CH = 128
N_CORES = 8
BPC = B // N_CORES
ALPHA, VTH = 0.3, 0.6
HW = H * W                      # 1024
PW = H + 1                      # 33: shared zero column between padded rows
XOFF = PW + 1                   # buf[XOFF + 33r + c] = padded cell (r, c)
XW = 1157                       # sbuf row: covers flat idx -34 .. 1122
XLEN = XW + PW + 1              # host buffer, so +33-shifted reads stay in range
NY = 24                         # y-tile ring size
YW = PW * W                     # 1056: y tiles carry a 33rd junk row so the
                                # three 11-row chunks drain in ONE ACT op each

# output row chunks (psum bank holds 512 f32: rows*33 <= 512); the last
# chunk computes a junk row 32 (reads only zero padding) for uniformity
CHUNKS = [(0, 11), (11, 22), (22, 33)]
PAIR_A = [-33, -32, -1]         # xm windows: tap a (lower) + tap a+33 (upper)
XS_A = 33                       # xs window: tap 33 (lower) + tap 34 (upper)
SINGLE_A = -34                  # K=64 single tap
CORR_A = PAIR_A + [SINGLE_A, 33, 34]   # DR windows (last 3 lower-half only)
S_XLO, S_WLO, S_WHI = 10, 16, 6  # fp8 scales: xlo*2^10, wlo*2^16, whi*2^6
TAILD = 760                      # tail column split: DVE gets [0:760], Pool rest


def _build_program():
    nc = bacc.Bacc("TRN2", target_bir_lowering=False, debug=False,
                   num_devices=N_CORES)

    xm_d = nc.dram_tensor("xm", [BPC, T, 128, XW], F16,
                          kind="ExternalInput").ap()
    xs_d = nc.dram_tensor("xs", [BPC, T, 128, XW], F16,
                          kind="ExternalInput").ap()
    xc_d = nc.dram_tensor("xc", [BPC, T, 128, 2, XW], FP8,
                          kind="ExternalInput").ap()
    wmain_d = nc.dram_tensor("wmain", [128, 5 * 128], F16,
                             kind="ExternalInput").ap()
    wcorr_d = nc.dram_tensor("wcorr", [128, 6, 2, 128], FP8,
                             kind="ExternalInput").ap()
    bias_d = nc.dram_tensor("biasv", [128, 1], F32, kind="ExternalInput").ap()
    w1t_d = nc.dram_tensor("w1t", [T, 5], F32, kind="ExternalInput").ap()
    w2t_d = nc.dram_tensor("w2t", [5, T], F32, kind="ExternalInput").ap()
    ident_d = nc.dram_tensor("ident", [128, 128], F32, kind="ExternalInput").ap()
    spk = nc.dram_tensor("spk", [BPC, T, CH, HW], FP8,
                         kind="ExternalOutput").ap()

    with tile.TileContext(nc) as tc:
        with tc.tile_pool(name="sb", bufs=1) as P1, \
             tc.tile_pool(name="scr", bufs=2) as P2, \
             tc.tile_pool(name="so", bufs=3) as P3, \
             tc.tile_pool(name="ps", bufs=1, space="PSUM") as PP:

            wmain = P1.tile([128, 5 * 128], F16, tag="wmain", name="wmain")
            nc.sync.dma_start(wmain[:], wmain_d[:])
            wcorr = P1.tile([128, 6, 2, 128], FP8, tag="wcorr", name="wcorr")
            nc.sync.dma_start(wcorr[:], wcorr_d[:])
            biast = P1.tile([128, 1], F32, tag="biasv", name="biasv")
            nc.sync.dma_start(biast[:], bias_d[:])
            w1t_s = P1.tile([T, 5], F32, tag="w1t", name="w1t")
            nc.sync.dma_start(w1t_s[:], w1t_d[:])
            w2t_s = P1.tile([5, T], F32, tag="w2t", name="w2t")
            nc.sync.dma_start(w2t_s[:], w2t_d[:])
            ident = P1.tile([128, 128], F32, tag="ident", name="ident")
            nc.sync.dma_start(ident[:], ident_d[:])
            ones_t = P1.tile([1, 128], F32, tag="ones", name="ones")
            nc.vector.memset(ones_t[:], 1.0)

            ys = [P1.tile([128, YW], F32, tag=f"y{i}", name=f"y{i}")
                  for i in range(NY)]
            xms = [P1.tile([128, XW], F16, tag=f"xm{i}", name=f"xm{i}")
                   for i in range(3)]
            xss = [P1.tile([128, XW], F16, tag=f"xs{i}", name=f"xs{i}")
                   for i in range(3)]
            xcs = [P1.tile([128, 2, XW], FP8, tag=f"xc{i}", name=f"xc{i}")
                   for i in range(3)]
            g_t = P1.tile([128, HW], F32, tag="g", name="g")
            mscr = P1.tile([128, HW], F32, tag="mscr", name="mscr")
            # per-frame stats: yA sum (incl. junk row 32; its and yB's tiny
            # contributions to the attention avg are negligible) + max
            s_st = [P1.tile([128, 2 * T], F32, tag=f"S{s}", name=f"S{s}")
                    for s in range(BPC)]
            # per-step scalars: bc0 = a*att[t-1]/att[t], -thr, +thr
            bc = [P1.tile([128, 3 * T], F32, tag=f"bc{s}", name=f"bc{s}")
                  for s in range(BPC)]

            def conv_frame(s, t):
                f = s * T + t
                xm = xms[f % 3]
                nc.sync.dma_start(xm[:], xm_d[s, t])
                xs = xss[f % 3]
                nc.sync.dma_start(xs[:], xs_d[s, t])
                xc = xcs[f % 3]
                nc.scalar.dma_start(xc[:], xc_d[s, t])

                y = ys[f % NY]
                S = s_st[s]
                psA = PP.tile([128, 3, 512], F32, tag="psA", name="psA")
                psB = PP.tile([128, 3, 512], F32, tag="psB", name="psB")
                # all psA matmuls first, then all psB DRs: frame f's psA
                # drain overlaps f's DR phase, f's psB drain overlaps f+1's
                # psA phase, so single-buffered psum tiles never stall PE.
                for c, (r0, r1) in enumerate(CHUNKS):
                    n = (r1 - r0) * PW
                    wbase = XOFF + r0 * PW
                    for j, a in enumerate(PAIR_A):
                        nc.tensor.matmul(
                            psA[:, c, 0:n], wmain[:, j * 128:(j + 1) * 128],
                            xm[:, wbase + a:wbase + a + n],
                            start=(j == 0), stop=False)
                    nc.tensor.matmul(
                        psA[:, c, 0:n], wmain[:, 3 * 128:4 * 128],
                        xs[:, wbase + XS_A:wbase + XS_A + n],
                        start=False, stop=False)
                    nc.tensor.matmul(
                        psA[:, c, 0:n], wmain[0:64, 4 * 128:5 * 128],
                        xm[0:64, wbase + SINGLE_A:wbase + SINGLE_A + n],
                        start=False, stop=True)
                for c, (r0, r1) in enumerate(CHUNKS):
                    n = (r1 - r0) * PW
                    wbase = XOFF + r0 * PW
                    for j, a in enumerate(CORR_A):
                        nc.tensor.matmul(
                            psB[:, c, 0:n], wcorr[:, j, :, :],
                            xc[:, :, wbase + a:wbase + a + n],
                            perf_mode=DR, start=(j == 0), stop=(j == 5))
                # interior views [p][chunk][row][col 0:32] (col 32 = pad):
                # ONE ACT drain per psum for all 3 chunks (banks adjacent)
                pAv = psA[:, :, 0:11 * PW].rearrange(
                    "p c (r w) -> p c r w", w=PW)[:, :, :, 0:32]
                pBv = psB[:, :, 0:11 * PW].rearrange(
                    "p c (r w) -> p c r w", w=PW)[:, :, :, 0:32]
                yA = P2.tile([128, YW], F32, tag="yA", name="yA")
                yA4 = yA.rearrange("p (c r w) -> p c r w", c=3, w=W)
                nc.scalar.activation(yA4[:], pAv, AF.Identity,
                                     bias=biast[:, 0:1],
                                     accum_out=S[:, t:t + 1])
                yB = P2.tile([128, YW], F32, tag="yB", name="yB")
                yB4 = yB.rearrange("p (c r w) -> p c r w", c=3, w=W)
                nc.scalar.activation(yB4[:], pBv, AF.Copy,
                                     scale=2.0 ** -S_WLO)
                nc.gpsimd.tensor_tensor(y[:], yA[:], yB[:], op=OP.add)
                nc.vector.tensor_scalar(
                    mscr[:], y[:, 0:HW], -3.0e38, None, op0=OP.max,
                    op1=OP.max, accum_out=S[:, T + t:T + t + 1])

            def attention(s):
                S = s_st[s]
                psTs = PP.tile([T, 128], F32, tag="pT0", name="psTs")
                psTm = PP.tile([T, 128], F32, tag="pT1", name="psTm")
                nc.tensor.transpose(psTs[:], S[:, 0:T], ident[:])
                nc.tensor.transpose(psTm[:], S[:, T:2 * T], ident[:])
                att_in = P2.tile([T, 2], F32, tag="att_in", name="att_in")
                tmp = P2.tile([T, 1], F32, tag="att_tmp", name="att_tmp")
                nc.vector.reduce_sum(tmp[:], psTs[:], axis=mybir.AxisListType.X)
                nc.vector.tensor_scalar_mul(att_in[:, 0:1], tmp[:],
                                            1.0 / (CH * HW))
                nc.vector.reduce_max(att_in[:, 1:2], psTm[:],
                                     axis=mybir.AxisListType.X)
                ps5 = PP.tile([5, 2], F32, tag="pT0", name="ps5")
                nc.tensor.matmul(ps5[:], w1t_s[:], att_in[:], start=True,
                                 stop=True)
                h5 = P2.tile([5, 2], F32, tag="h5", name="h5")
                nc.scalar.activation(h5[:], ps5[:], AF.Relu)
                ps20 = PP.tile([T, 2], F32, tag="pT1", name="ps20")
                nc.tensor.matmul(ps20[:], w2t_s[:], h5[:], start=True, stop=True)
                a20 = P2.tile([T, 2], F32, tag="a20", name="a20")
                nc.scalar.activation(a20[:], ps20[:], AF.Copy)
                attp = P2.tile([T, 1], F32, tag="attp", name="attp")
                nc.vector.tensor_tensor(attp[:], a20[:, 0:1], a20[:, 1:2],
                                        op=OP.add)
                # sigmoid via exp + reciprocal; ab2 = [att | 1/att]
                expz = P2.tile([T, 1], F32, tag="expz", name="expz")
                nc.scalar.activation(expz[:], attp[:], AF.Exp, scale=-1.0)
                ab2 = P2.tile([T, 2], F32, tag="ab2", name="ab2")
                nc.vector.tensor_scalar_add(ab2[:, 1:2], expz[:], 1.0)
                nc.vector.reciprocal(ab2[:, 0:1], ab2[:, 1:2])
                # transpose att / 1/att to free-dim [1, T] vectors (PSUM
                # reads must start at partition 0, so two separate tiles)
                psTa = PP.tile([1, T], F32, tag="pT0", name="psTa")
                nc.tensor.transpose(psTa[:], ab2[:, 0:1], ident[0:T, 0:T])
                psTb = PP.tile([1, T], F32, tag="pT1", name="psTb")
                nc.tensor.transpose(psTb[:], ab2[:, 1:2], ident[0:T, 0:T])
                attf = P2.tile([1, T + 1], F32, tag="attf", name="attf")
                nc.vector.tensor_copy(attf[0:1, 1:T + 1], psTa[0:1, 0:T])
                nc.vector.tensor_copy(attf[0:1, 0:1], psTa[0:1, 0:1])
                rhs_bc = P2.tile([1, 3 * T], F32, tag="rhs_bc", name="rhs_bc")
                nc.vector.scalar_tensor_tensor(
                    rhs_bc[0:1, 0:T], attf[0:1, 0:T], ALPHA, psTb[0:1, 0:T],
                    op0=OP.mult, op1=OP.mult)
                nc.vector.tensor_scalar_mul(rhs_bc[0:1, T:2 * T],
                                            psTb[0:1, 0:T], -VTH)
                nc.vector.tensor_scalar_mul(rhs_bc[0:1, 2 * T:3 * T],
                                            psTb[0:1, 0:T], VTH)
                ps_bc = PP.tile([128, 3 * T], F32, tag="pT0", name="ps_bc")
                nc.tensor.matmul(ps_bc[:], ones_t[:], rhs_bc[:], start=True,
                                 stop=True)
                nc.scalar.activation(bc[s][:], ps_bc[:], AF.Copy)

            def scan_step(s, t, tail=False):
                f = s * T + t
                yv = ys[f % NY][:, 0:HW]
                if t == 0:
                    vsrc = yv
                else:
                    v = P2.tile([128, HW], F32, tag="v", name="v")
                    vsrc = v[:]
                spm = P3.tile([128, HW], FP8, tag="spm", name="spm")
                if tail:
                    # column-split the serial chain: DVE runs [0:TAILD] with
                    # STT ops; Pool (no STT support) runs [TAILD:] with a
                    # TS + 2xTT chain that keeps g pre-scaled by the next
                    # step's bc0 so the v-update is a plain TT add. Spike out
                    # on ACT (idle in the tail), split to decouple the chains.
                    slD = slice(0, TAILD)
                    slP = slice(TAILD, HW)
                    if t != 0:
                        nc.vector.scalar_tensor_tensor(
                            vsrc[:, slD], g_t[:, slD], bc[s][:, t:t + 1],
                            yv[:, slD], op0=OP.mult, op1=OP.add)
                        nc.gpsimd.tensor_tensor(vsrc[:, slP], g_t[:, slP],
                                                yv[:, slP], op=OP.add)
                    nc.scalar.activation(
                        spm[:, slD], vsrc[:, slD], AF.Sign,
                        bias=bc[s][:, T + t:T + t + 1])
                    nc.scalar.activation(
                        spm[:, slP], vsrc[:, slP], AF.Sign,
                        bias=bc[s][:, T + t:T + t + 1])
                    if t != T - 1:
                        nc.vector.scalar_tensor_tensor(
                            g_t[:, slD], vsrc[:, slD],
                            bc[s][:, 2 * T + t:2 * T + t + 1],
                            vsrc[:, slD], op0=OP.is_lt, op1=OP.mult)
                        # gp = [v < thr] * bc0[t+1]; g = gp * v
                        gp = P2.tile([128, HW - TAILD], F32, tag="gp",
                                     name="gp")
                        nc.gpsimd.tensor_scalar(
                            gp[:], vsrc[:, slP],
                            bc[s][:, 2 * T + t:2 * T + t + 1],
                            bc[s][:, t + 1:t + 2], op0=OP.is_lt, op1=OP.mult)
                        nc.gpsimd.tensor_tensor(g_t[:, slP], gp[:],
                                                vsrc[:, slP], op=OP.mult)
                else:
                    # overlapped with conv: v/gate on DVE, spike on ACT
                    # (its sign table; DVE is the scarcer engine here)
                    if t != 0:
                        nc.vector.scalar_tensor_tensor(
                            vsrc, g_t[:], bc[s][:, t:t + 1], yv,
                            op0=OP.mult, op1=OP.add)
                    nc.scalar.activation(
                        spm[:], vsrc, AF.Sign,
                        bias=bc[s][:, T + t:T + t + 1])
                    if t != T - 1:
                        nc.vector.scalar_tensor_tensor(
                            g_t[:], vsrc,
                            bc[s][:, 2 * T + t:2 * T + t + 1],
                            vsrc, op0=OP.is_lt, op1=OP.mult)
                nc.scalar.dma_start(spk[s, t], spm[:])

            for t in range(T):
                conv_frame(0, t)
            attention(0)
            for t in range(T):
                scan_step(0, t)
                conv_frame(1, t)
            attention(1)
            for t in range(T):
                scan_step(1, t, tail=True)

    nc.compile()
    return nc


def _prep_host_inputs(conv_w, conv_b, mlp_w1, mlp_w2):
    whi = conv_w.astype(np.float16).astype(np.float32)   # [128,64,3,3]
    wlo = (conv_w - whi).astype(np.float32)

    def tapT(w, o):
        # o = dy*33 + dx with dy,dx in {-1,0,1}
        dy = int(np.round(o / PW))
        dx = o - PW * dy
        assert dy in (-1, 0, 1) and dx in (-1, 0, 1), o
        return np.ascontiguousarray(w[:, :, dy + 1, dx + 1].T)

    wmain = np.zeros((128, 5 * 128), np.float32)
    for j, a in enumerate(PAIR_A):
        wmain[0:64, j * 128:(j + 1) * 128] = tapT(whi, a)
        wmain[64:128, j * 128:(j + 1) * 128] = tapT(whi, a + PW)
    wmain[0:64, 3 * 128:4 * 128] = tapT(whi, 33)
    wmain[64:128, 3 * 128:4 * 128] = tapT(whi, 34)
    wmain[0:64, 4 * 128:5 * 128] = tapT(whi, SINGLE_A)

    wlo16 = wlo * np.float32(2.0 ** S_WLO)
    whi6 = whi * np.float32(2.0 ** S_WHI)
    wcorr = np.zeros((128, 6, 2, 128), np.float32)
    for j, a in enumerate(CORR_A):
        wcorr[0:64, j, 0, :] = tapT(wlo16, a)
        wcorr[0:64, j, 1, :] = tapT(whi6, a)
        if j < 3:
            wcorr[64:128, j, 0, :] = tapT(wlo16, a + PW)
            wcorr[64:128, j, 1, :] = tapT(whi6, a + PW)
    return {
        "wmain": wmain.astype(np.float16),
        "wcorr": wcorr.astype(E4),
        "biasv": np.ascontiguousarray(conv_b.reshape(128, 1), np.float32),
        "w1t": np.ascontiguousarray(mlp_w1.T).astype(np.float32),
        "w2t": np.ascontiguousarray(mlp_w2.T).astype(np.float32),
        "ident": np.eye(128, dtype=np.float32),
    }


def _shard_inputs(data):
    """data [BPC,T,64,32,32] -> xm/xs [BPC,T,128,XW] f16, xc [...,2,XW] e4m3.

    Flat PW=33 layout: buf[XOFF + 33r + c] = padded(r, c); interior r,c in
    [0,32); all pad cells (r or c = -1 or 32) collapse onto shared zeros.
    """
    lead = data.shape[:2]
    xp = np.zeros(lead + (CIN, XLEN), np.float32)
    idx = XOFF + PW * np.arange(H)[:, None] + np.arange(W)[None, :]
    xp[..., idx.reshape(-1)] = data.reshape(lead + (CIN, HW))
    xhi = xp.astype(np.float16)
    xlo10 = ((xp - xhi.astype(np.float32)) * np.float32(2.0 ** S_XLO))
    xm = np.empty(lead + (128, XW), np.float16)
    xm[..., 0:64, :] = xhi[..., 0:XW]
    xm[..., 64:128, :] = xhi[..., PW:PW + XW]
    xs = np.empty(lead + (128, XW), np.float16)
    xs[..., 0:64, :] = xhi[..., 0:XW]
    xs[..., 64:128, :] = xhi[..., 1:1 + XW]
    xhi8 = xhi.astype(E4)
    xlo8 = xlo10.astype(E4)
    xc = np.empty(lead + (128, 2, XW), E4)
    xc[..., 0:64, 0, :] = xhi8[..., 0:XW]
    xc[..., 0:64, 1, :] = xlo8[..., 0:XW]
    xc[..., 64:128, 0, :] = xhi8[..., PW:PW + XW]
    xc[..., 64:128, 1, :] = xlo8[..., PW:PW + XW]
    return xm, xs, xc


_CACHED = {}


def make_in_maps(data, conv_w, conv_b, mlp_w1, mlp_w2):
    data = np.ascontiguousarray(data, np.float32)
    common = _prep_host_inputs(np.asarray(conv_w, np.float32),
                               np.asarray(conv_b, np.float32),
                               np.asarray(mlp_w1, np.float32),
                               np.asarray(mlp_w2, np.float32))
    in_maps = []
    for c in range(N_CORES):
        m = dict(common)
        xm, xs, xc = _shard_inputs(data[c * BPC:(c + 1) * BPC])
        m["xm"] = xm
        m["xs"] = xs
        m["xc"] = xc
        in_maps.append(m)
    return in_maps


def kernel(data, conv_w, conv_b, mlp_w1, mlp_w2):
    if "prog" not in _CACHED:
        _CACHED["prog"] = _build_program()
    nc = _CACHED["prog"]
    in_maps = make_in_maps(data, conv_w, conv_b, mlp_w1, mlp_w2)
    res = run_bass_kernel_spmd(nc, in_maps, list(range(N_CORES)))
    out = np.concatenate(
        [np.asarray(res.results[c]["spk"]).astype(np.float32)
         for c in range(N_CORES)], axis=0)
    out = (out > 0).astype(np.float32)
    return out.reshape(B, T, CH, H, W)
